# revision 4
# baseline (speedup 1.0000x reference)
"""Trainium2 Bass kernel for nn_AGCBlock.

Math: the reference's Sa_GC spatial pool applies log_softmax over a
singleton axis (shape [N, 1, KK]), which is exactly zero, so the pooled
context is exactly zero for every patch.  The channel_add branch then
reduces to a constant vector

    t    = b1                      (context @ w1.T == 0)
    tn   = LN(t) * gamma + beta ; relu
    term = w2 @ tn + b2            # [64], independent of x and the patch

and out_p = patches + term.  fold(unfold(x) + term)/fold(unfold(1)) =
x + term (the overlap counts cancel, every pixel is covered since
stride 7 < kernel 15).  So the kernel is a memory-bound broadcast add:

    out[b, c, h, w] = x[b, c, h, w] + term[c]

Distribution: data-parallel over channels -- core i handles channels
[8i, 8i+8) (contiguous slice of x, zero-copy shard).  Each core computes
its 8 entries of `term` on device (tiny LN + matmul chain using its w2/b2
shard) and streams its 8 MiB x-shard through SBUF adding term per
partition.  Layout per core: [8, 512, 512] viewed as [128, 16384] with
partition p <-> (channel p//16, row-block p%16), so the per-partition
bias is term repeated 16x.
"""

import numpy as np
from contextlib import ExitStack

import concourse.bass as bass
import concourse.tile as tile
from concourse import bacc, mybir
from concourse.bass_utils import run_bass_kernel_spmd

B, C, H, W = 1, 64, 512, 512
NCORES = 8
CPC = C // NCORES          # 8 channels per core
P = 128                    # SBUF partitions
HH = P // CPC              # 16 row-blocks per channel
FREE = (H // HH) * W       # 32 * 512 = 16384 elements per partition
TS = 2048                  # column tile -> 1 MiB per DMA
NT = FREE // TS
PLANES = 32
EPS = 1e-5

_nc_cache = []


def _build():
    f32 = mybir.dt.float32
    nc = bacc.Bacc("TRN2", target_bir_lowering=False, debug=False,
                   num_devices=NCORES)

    x_h = nc.declare_dram_parameter("x", [P, FREE], f32, isOutput=False)
    b1_h = nc.declare_dram_parameter("b1", [1, PLANES], f32, isOutput=False)
    g_h = nc.declare_dram_parameter("gamma", [1, PLANES], f32, isOutput=False)
    be_h = nc.declare_dram_parameter("beta", [1, PLANES], f32, isOutput=False)
    w2_h = nc.declare_dram_parameter("w2", [CPC, PLANES], f32, isOutput=False)
    b2_h = nc.declare_dram_parameter("b2", [CPC, 1], f32, isOutput=False)
    out_h = nc.declare_dram_parameter("out", [P, FREE], f32, isOutput=True)

    scratch = nc.dram_tensor("term_scratch", [P], f32)

    with tile.TileContext(nc) as tc:
        with ExitStack() as ctx:
            singles = ctx.enter_context(tc.tile_pool(name="singles", bufs=1))
            psum = ctx.enter_context(
                tc.tile_pool(name="psum", bufs=1, space="PSUM"))
            xpool = ctx.enter_context(tc.tile_pool(name="x", bufs=4))

            b1r = singles.tile([1, PLANES], f32)
            nc.sync.dma_start(b1r[:], b1_h[:])
            gr = singles.tile([1, PLANES], f32)
            nc.sync.dma_start(gr[:], g_h[:])
            ber = singles.tile([1, PLANES], f32)
            nc.sync.dma_start(ber[:], be_h[:])
            w2s = singles.tile([CPC, PLANES], f32)
            nc.sync.dma_start(w2s[:], w2_h[:])
            b2c = singles.tile([CPC, 1], f32)
            nc.sync.dma_start(b2c[:], b2_h[:])

            ones = singles.tile([1, CPC], f32)
            nc.vector.memset(ones[:], 1.0)

            # ---- LayerNorm(b1) * gamma + beta, relu  (all on partition 0)
            s1 = singles.tile([1, 1], f32)
            nc.vector.reduce_sum(s1[:], b1r[:], axis=mybir.AxisListType.X)
            sq = singles.tile([1, PLANES], f32)
            nc.vector.tensor_mul(sq[:], b1r[:], b1r[:])
            s2 = singles.tile([1, 1], f32)
            nc.vector.reduce_sum(s2[:], sq[:], axis=mybir.AxisListType.X)
            mu = singles.tile([1, 1], f32)
            nc.vector.tensor_scalar_mul(mu[:], s1[:], 1.0 / PLANES)
            msq = singles.tile([1, 1], f32)
            nc.vector.tensor_mul(msq[:], mu[:], mu[:])
            var = singles.tile([1, 1], f32)
            nc.vector.tensor_scalar_mul(var[:], s2[:], 1.0 / PLANES)
            nc.vector.tensor_sub(var[:], var[:], msq[:])
            nc.vector.tensor_scalar_add(var[:], var[:], EPS)
            std = singles.tile([1, 1], f32)
            nc.scalar.sqrt(std[:], var[:])
            inv = singles.tile([1, 1], f32)
            nc.vector.reciprocal(inv[:], std[:])

            xm = singles.tile([1, PLANES], f32)
            nc.vector.tensor_scalar_sub(xm[:], b1r[:], mu[:])
            nc.vector.tensor_scalar_mul(xm[:], xm[:], inv[:])
            nc.vector.tensor_mul(xm[:], xm[:], gr[:])
            nc.vector.tensor_add(xm[:], xm[:], ber[:])
            tn = singles.tile([1, PLANES], f32)
            nc.vector.tensor_scalar_max(tn[:], xm[:], 0.0)

            # ---- term8 = w2_shard @ tn + b2_shard  ([CPC, 1])
            # broadcast tn to CPC partitions via ones-outer-product matmul
            pb = psum.tile([CPC, PLANES], f32)
            nc.tensor.matmul(pb[:], ones[:], tn[:])
            prod = singles.tile([CPC, PLANES], f32)
            nc.vector.tensor_mul(prod[:], w2s[:], pb[:])
            term8 = singles.tile([CPC, 1], f32)
            nc.vector.reduce_sum(term8[:], prod[:], axis=mybir.AxisListType.X)
            nc.vector.tensor_add(term8[:], term8[:], b2c[:])

            # ---- replicate to [P, 1]: term128[p] = term8[p // HH]
            t16 = singles.tile([CPC, HH], f32)
            nc.vector.tensor_copy(t16[:, 0:1], term8[:])
            k = 1
            while k < HH:
                kk = min(k, HH - k)
                nc.vector.tensor_copy(t16[:, k:k + kk], t16[:, 0:kk])
                k += kk
            nc.sync.dma_start(scratch[:], t16[:])
            term128 = singles.tile([P, 1], f32)
            nc.sync.dma_start(term128[:], scratch[:])

            # ---- main stream: out = x + term128 (per-partition bias)
            for j in range(NT):
                t = xpool.tile([P, TS], f32)
                nc.sync.dma_start(t[:], x_h[:, j * TS:(j + 1) * TS])
                nc.vector.tensor_scalar_add(t[:], t[:], term128[:])
                nc.sync.dma_start(out_h[:, j * TS:(j + 1) * TS], t[:])

    nc.finalize()
    return nc


def kernel(x, w_mask, b_mask, w1, b1, gamma, beta, w2, b2):
    x = np.ascontiguousarray(np.asarray(x, dtype=np.float32))
    b1 = np.asarray(b1, dtype=np.float32).reshape(1, PLANES)
    gamma = np.asarray(gamma, dtype=np.float32).reshape(1, PLANES)
    beta = np.asarray(beta, dtype=np.float32).reshape(1, PLANES)
    w2 = np.asarray(w2, dtype=np.float32).reshape(C, PLANES)
    b2 = np.asarray(b2, dtype=np.float32).reshape(C, 1)

    if not _nc_cache:
        _nc_cache.append(_build())
    nc = _nc_cache[0]

    xs = x.reshape(C, H, W)
    in_maps = []
    for i in range(NCORES):
        c0 = i * CPC
        in_maps.append({
            "x": xs[c0:c0 + CPC].reshape(P, FREE),
            "b1": b1,
            "gamma": gamma,
            "beta": beta,
            "w2": w2[c0:c0 + CPC],
            "b2": b2[c0:c0 + CPC],
        })

    res = run_bass_kernel_spmd(nc, in_maps, core_ids=list(range(NCORES)))
    out = np.concatenate(
        [res.results[i]["out"].reshape(CPC, H, W) for i in range(NCORES)],
        axis=0,
    )
    return out.reshape(B, C, H, W)
